# revision 12
# baseline (speedup 1.0000x reference)
"""Trainium2 Bass kernel for nn_InpaintContextAttentionUnit.

Per-sample computation (B=8 samples -> 1 per NeuronCore):
  fm [512,512,16] -> avgpool(64x2) -> pooled [8,256,16]
  -> two masked 3x3 convs (middle row / middle col of kernel zeroed) + bias + relu
  -> bilinear upsample back to [512,512,16] (separable; half-pixel centers, edge clamp)
  -> out [512,512,48] = concat(fm, fm - row_up, fm - col_up)

Design v5 — fully pipelined; compute balanced across DVE/ACT/GpSimd:
  - out tile t only needs pooled conv rows < 2t+3, so output streaming
    starts after 2 input tiles (~30us) and HBM never idles
  - W-upsample into a PER-TILE rotating scratch rwT (recompute rows
    [0,nhi) each tile): kills all cross-tile write-after-read hazards
    (rounds into a shared rw serialized behind DMA-paced H-up matmuls),
    and par-planar layout (par, f, xp) makes the STT writes contiguous
    (measured 4.7us strided vs ~2us contiguous; DVE 2x mode needs
    16-bit step-1)
  - H-up matmul rhs (par, f4, xp64) slices; psum per (q,b,fh,i) 1 bank;
    subtracts on DVE (GpSimd cannot read PSUM); DVE queue holds ONLY
    stt+subs (~25us/tile), pair-adds and half the casts go to GpSimd,
    pass-through copies/psum-casts/relus to ACT -- each engine stays
    under the 27.7us/tile DMA pace
  - pooling: W-pair-add fused with (x,c)->(c,xp) transpose (DVE), one
    [128,2]x[128,512] PE matmul per eighth, psum casts on ACT
  - pooled-row edge duplication via tiny DMAs in the bounce (no engine)
  - quarter-granularity loads and casts (ACT/DVE/ACT/GpSimd) keep the
    load stream dense with fmf bufs=3 at 8 KB
All constant matrices are precomputed on host and passed as extra inputs.
"""

import numpy as np
import ml_dtypes

H, W, C, F = 512, 512, 16, 16
NPOOL = 8
WP = W // 2  # 256
CH_OUT = 3 * C  # 48

_cache = {}


def _host_consts(kernel, bias):
    """Build host-side constant matrices (bf16 for the PE-side constants)."""
    bf = ml_dtypes.bfloat16
    # pooling weights: [128, 2], 1/128 (exact in bf16) where row block matches
    poolw = np.zeros((128, 2), np.float32)
    poolw[:64, 0] = 1.0 / 128.0
    poolw[64:, 1] = 1.0 / 128.0
    # H-upsample matrix: hup[n, y] = weight of pooled row n for output row y,
    # scaled by 0.75 (the W-upsample major tap; k/64*0.75 = 3k/256 exact in bf16)
    hup = np.zeros((NPOOL, H), np.float32)
    scale = H // NPOOL
    for y in range(H):
        yf = (y + 0.5) / scale - 0.5
        i0 = int(np.floor(yf))
        w = yf - i0
        hup[min(max(i0, 0), NPOOL - 1), y] += 1.0 - w
        hup[min(max(i0 + 1, 0), NPOOL - 1), y] += w
    hup *= 0.75
    hup2 = np.zeros((40, H), np.float32)
    hup2[0:8] = hup
    hup2[32:40] = hup  # col-branch copy at base partition 32; rows 8-15 stay zero
    # conv taps, stacked over dwp on 48 partitions (partition 16g+c holds the
    # dwp=g-1 shifted pooled copy). One matmul slot per (branch, dn):
    #   branch 0 (row conv): kernel[dn+1, dwp+1]; slots 0,1 for dn=-1,+1
    #   branch 1 (col conv): kernel[dwp+1, dn+1], dwp=0 block zero; slots 2-4
    kt = np.zeros((48, 5 * 16), np.float32)  # [(g,c), slot*16+f]
    for g in range(3):
        for s, dn in enumerate((-1, 1)):
            kt[16 * g:16 * (g + 1), s * 16:(s + 1) * 16] = kernel[dn + 1, g]
        for s, dn in enumerate((-1, 0, 1)):
            if g != 1:
                kt[16 * g:16 * (g + 1), (2 + s) * 16:(3 + s) * 16] = \
                    kernel[g, dn + 1]
    bias2 = np.ascontiguousarray(bias.reshape(16, 1)).astype(np.float32)
    return (poolw.astype(bf), hup2.astype(bf), kt.astype(bf), bias2, None, None)


def _build_program(compile=True):
    import concourse.bass as bass
    import concourse.bacc as bacc
    import concourse.mybir as mybir
    import concourse.tile as tile

    dt = mybir.dt.float32
    db = mybir.dt.bfloat16
    nc = bacc.Bacc()

    fm_d = nc.declare_dram_parameter("feature_map", [H, W, C], dt, isOutput=False)
    poolw_d = nc.declare_dram_parameter("poolw", [128, 2], db, isOutput=False)
    hup_d = nc.declare_dram_parameter("hup", [40, H], db, isOutput=False)
    ktaps_d = nc.declare_dram_parameter("ktaps", [48, 80], db, isOutput=False)
    bias_d = nc.declare_dram_parameter("bias2", [16, 1], dt, isOutput=False)
    out_d = nc.declare_dram_parameter("out", [H, W, CH_OUT], dt, isOutput=True)

    # matmul slots per branch: (slot, dn)
    slots_by_branch = [[(0, -1), (1, 1)], [(2, -1), (3, 0), (4, 1)]]

    with tile.TileContext(nc) as tc:
        with (
            tc.tile_pool(name="consts", bufs=1) as cpool,
            tc.tile_pool(name="persist", bufs=1) as ppool,
            tc.tile_pool(name="work", bufs=1) as wpool,
            tc.tile_pool(name="dram", bufs=1, space="DRAM") as dpool,
            tc.tile_pool(name="psall", bufs=1, space="PSUM") as psall,
        ):
            poolw_t = cpool.tile([128, 2], db)
            hup_t = cpool.tile([40, H], db)
            ktaps_t = cpool.tile([48, 80], db)
            bias_t = cpool.tile([16, 1], dt)

            # persistent bf16 fm copy: [128, (4 t, 512 x, 16 c)]
            fmb_t = ppool.tile([128, 4 * W * C], db)

            tpad_t = ppool.tile([48, 10 * 258], db)
            t48 = tpad_t[:].rearrange("p (n w) -> p n w", w=258)
            conv_t = ppool.tile([16, 2 * NPOOL * WP], db)
            rop_t = ppool.tile([40, 16 * 258], db)
            rop3 = rop_t[:].rearrange("p (f w) -> p f w", w=258)
            fmb4 = fmb_t[:].rearrange("p (t x c) -> p t x c", t=4, c=16)
            # pair-add views: (t, par, c, xp) with par the W-pair index
            fmbp = fmb_t[:].rearrange(
                "p (t xp par c) -> p t par c xp", t=4, par=2, c=16)
            # sub in0 view: (t, xp, par, c)
            fmbq = fmb_t[:].rearrange(
                "p (t xp par c) -> p t xp par c", t=4, par=2, c=16)

            ncw_dram = dpool.tile([NPOOL, 16 * 258], db)
            nd3 = ncw_dram[:].rearrange("n (c w) -> n c w", w=258)
            ncwd3 = ncw_dram[:].rearrange("n (c w) -> c n w", w=258)
            conv_dram = dpool.tile([16, 2 * NPOOL * WP], db)
            cd4 = conv_dram[:].rearrange("f (b n w) -> b n f w", b=2, n=NPOOL)
            zsrc = hup_d[8:16, 0:16]  # [8, 16] zeros

            # zero-fill t48 halo rows (0 and 9) and rop (rows 8-31 are read
            # by the partition-spanning W-up STTs); GpSimd is otherwise idle
            nc.gpsimd.memset(tpad_t[:], 0.0)
            nc.gpsimd.memset(rop_t[:], 0.0)

            def load_tile(t):
                fmfs = []
                for qu in range(4):
                    fmf = wpool.tile([128, W * C // 4], dt, tag="fmf", bufs=3,
                                     name=f"fmf{t}{qu}")
                    fmf3 = fmf[:].rearrange("p (x c) -> p x c", c=C)
                    nc.sync.dma_start(
                        out=fmf3,
                        in_=fm_d[128 * t:128 * (t + 1),
                                 128 * qu:128 * (qu + 1)])
                    fmfs.append(fmf)
                return fmfs

            def cast_tile(t, fmfs):
                qsz = W * C // 4
                engs = [nc.scalar, nc.gpsimd, nc.scalar, nc.gpsimd]
                for qu in range(4):
                    dst = fmb_t[:, t * W * C + qu * qsz:
                                t * W * C + (qu + 1) * qsz]
                    if engs[qu] is nc.scalar:
                        nc.scalar.activation(
                            out=dst, in_=fmfs[qu][:],
                            func=mybir.ActivationFunctionType.Copy)
                    else:
                        engs[qu].tensor_copy(dst, fmfs[qu][:])

            def pool_tile(t):
                # W-pair add fused with the (x,c)->(c,xp) transpose (DVE),
                # then one [128p,2]x[128p,512] PE matmul per eighth with
                # contiguous rhs; f32->bf16 psum casts on ACT
                fmpT = wpool.tile([128, 16 * WP], db, tag="fmpT", bufs=1,
                                  name=f"fmpT{t}")
                fmpT3 = fmpT[:].rearrange("p (c xp) -> p c xp", xp=WP)
                nc.gpsimd.tensor_add(
                    fmpT3, fmbp[:, t, 0, :, :], fmbp[:, t, 1, :, :])
                stage = wpool.tile([2, WP * 16], db, tag="stage", bufs=1,
                                   name=f"stage{t}")
                for e in range(8):
                    ps = psall.tile([2, 512], dt, tag="pool", bufs=2,
                                    name=f"psp{t}{e}")
                    nc.tensor.matmul(
                        ps[:], poolw_t[:],
                        fmpT3[:, 2 * e:2 * (e + 1), :],
                        start=True, stop=True,
                    )
                    nc.scalar.activation(
                        out=stage[:, 512 * e:512 * (e + 1)], in_=ps[:],
                        func=mybir.ActivationFunctionType.Copy)
                # bounce to DRAM ((c, xp)-major already); read back 3
                # dwp-shifted copies with c on partitions (n rows shifted
                # +1 for the zero halo)
                stage3 = stage[:].rearrange("p (c xp) -> p c xp", xp=WP)
                nc.sync.dma_start(
                    out=nd3[2 * t:2 * t + 2, :, 1:257], in_=stage3)
                for g in range(3):
                    nc.sync.dma_start(
                        out=t48[16 * g:16 * (g + 1),
                                2 * t + 1:2 * t + 3, 1:257],
                        in_=ncwd3[:, 2 * t:2 * t + 2, g:g + 256])

            def conv_unit(b, n0, nn):
                # conv rows n0..n0+nn; dwp taps contracted via the
                # 48-partition stack, one accumulating matmul per dn
                ps = psall.tile([16, 2 * WP], dt, tag="conv", bufs=2,
                                name=f"psc{b}{n0}")
                slots = slots_by_branch[b]
                for k, (sl, dn) in enumerate(slots):
                    nc.tensor.matmul(
                        ps[:, 0:nn * WP],
                        ktaps_t[:, sl * 16:(sl + 1) * 16],
                        t48[:, n0 + dn + 1:n0 + dn + 1 + nn, 1:257],
                        start=(k == 0), stop=(k == len(slots) - 1),
                    )
                nc.scalar.activation(
                    out=conv_t[:, (b * NPOOL + n0) * WP:
                               (b * NPOOL + n0 + nn) * WP],
                    in_=ps[:, 0:nn * WP],
                    func=mybir.ActivationFunctionType.Relu,
                    bias=bias_t[:, 0:1],
                )

            def tail(b, nlo, nhi):
                # bounce conv rows [nlo,nhi) to rop [(b,n) parts, (f, wp)];
                # edge columns (pad clamp) duplicated by two tiny DMAs
                nc.sync.dma_start(
                    out=conv_dram[:, (b * NPOOL + nlo) * WP:
                                  (b * NPOOL + nhi) * WP],
                    in_=conv_t[:, (b * NPOOL + nlo) * WP:
                               (b * NPOOL + nhi) * WP])
                rows = slice(32 * b + nlo, 32 * b + nhi)
                nc.sync.dma_start(
                    out=rop3[rows, :, 1:257], in_=cd4[b][nlo:nhi])
                nc.sync.dma_start(
                    out=rop3[rows, :, 0:1], in_=cd4[b][nlo:nhi, :, 0:1])
                nc.sync.dma_start(
                    out=rop3[rows, :, 257:258],
                    in_=cd4[b][nlo:nhi, :, 255:256])

            def stt_tile(t):
                # W-upsample rows [0, nhi) of both branches into this
                # tile's scratch rwT, par-planar (par, f, xp) so writes are
                # contiguous; 0.75 folded into hup:
                #   rw[2k] = pad[k]/3 + pad[k+1]; rw[2k+1] = pad[k+2]/3 + pad[k+1]
                # Rows 8-31 compute junk from the zeroed rop, never read.
                nhi = min(8, 2 * t + 3)
                rwT = wpool.tile([40, 2 * 16 * WP], db, tag="rwT", bufs=2,
                                 name=f"rwT{t}")
                rwTv = rwT[:].rearrange("p (par f xp) -> p par f xp",
                                        par=2, f=16)
                span = slice(0, 32 + nhi)
                third = 1.0 / 3.0
                nc.vector.scalar_tensor_tensor(
                    out=rwTv[span, 0, :, :],
                    in0=rop3[span, :, 0:256],
                    scalar=third,
                    in1=rop3[span, :, 1:257],
                    op0=mybir.AluOpType.mult,
                    op1=mybir.AluOpType.add,
                )
                nc.vector.scalar_tensor_tensor(
                    out=rwTv[span, 1, :, :],
                    in0=rop3[span, :, 2:258],
                    scalar=third,
                    in1=rop3[span, :, 1:257],
                    op0=mybir.AluOpType.mult,
                    op1=mybir.AluOpType.add,
                )
                return rwTv

            def passB_tile(t, rwTv):
                # output rows 128t..128t+127 read only pooled conv rows
                # < nhi = 2t+3 (hup weights at rows >= nhi are zero; matmul
                # base partitions must be 0/32, so contract from pg)
                nhi = min(8, 2 * t + 3)
                for q in range(4):
                    outq = wpool.tile([128, 128 * CH_OUT], dt,
                                      tag="outq", bufs=2, name=f"oq{t}{q}")
                    outq3 = outq[:].rearrange("p (x ch) -> p x ch",
                                              ch=CH_OUT)
                    outq5 = outq[:].rearrange("p (xp par ch) -> p xp par ch",
                                              par=2, ch=CH_OUT)
                    fmq = fmb4[:, t, 128 * q:128 * (q + 1), :]
                    nc.scalar.activation(
                        out=outq3[:, :, 0:16], in_=fmq,
                        func=mybir.ActivationFunctionType.Copy,
                    )
                    for b in range(2):
                        pg = 32 * b
                        lhsT = hup_t[pg:pg + nhi, 128 * t:128 * (t + 1)]
                        for fh in range(2):
                            for i in range(2):
                                ps = psall.tile(
                                    [128, 512], dt, tag="up", bufs=4,
                                    name=f"psu{t}{q}{b}{fh}{i}")
                                nc.tensor.matmul(
                                    ps[:],
                                    lhsT,
                                    rwTv[pg:pg + nhi, :,
                                         8 * fh + 4 * i:8 * fh + 4 * (i + 1),
                                         64 * q:64 * (q + 1)],
                                    start=True, stop=True,
                                )
                                psx = ps[:].rearrange(
                                    "p (par f xp) -> p xp par f",
                                    par=2, f=4)
                                ch0 = 16 * (b + 1) + 8 * fh + 4 * i
                                nc.vector.tensor_sub(
                                    outq5[:, :, :, ch0:ch0 + 4],
                                    fmbq[:, t, 64 * q:64 * (q + 1), :,
                                         8 * fh + 4 * i:8 * fh + 4 * (i + 1)],
                                    psx)
                    nc.sync.dma_start(
                        out=out_d[128 * t:128 * (t + 1),
                                  128 * q:128 * (q + 1), :],
                        in_=outq3,
                    )

            # ---------------- pipelined schedule ----------------
            fmfs0 = load_tile(0)
            nc.sync.dma_start(out=poolw_t[:], in_=poolw_d[:])
            nc.sync.dma_start(out=hup_t[:], in_=hup_d[:])
            nc.sync.dma_start(out=ktaps_t[:], in_=ktaps_d[:])
            nc.sync.dma_start(out=bias_t[:], in_=bias_d[:])
            nc.sync.dma_start(out=nd3[:, :, 0:1], in_=zsrc)
            nc.sync.dma_start(out=nd3[:, :, 257:258], in_=zsrc)
            fmfs1 = load_tile(1)

            cast_tile(0, fmfs0)
            pool_tile(0)
            # conv row 0 only needs pooled rows 0-1 (tile 0)
            for b in range(2):
                conv_unit(b, 0, 1)
            cast_tile(1, fmfs1)
            pool_tile(1)
            fmfs2 = load_tile(2)

            # conv rows 1-2 (needs pooled rows 0-3), W-up rows 0-2
            for b in range(2):
                conv_unit(b, 1, 2)
                tail(b, 0, 3)
            rwTv0 = stt_tile(0)

            cast_tile(2, fmfs2)
            fmfs3 = load_tile(3)

            passB_tile(0, rwTv0)

            pool_tile(2)
            # conv rows 3-4 (needs pooled rows 2-5), W-up rows 0-4
            for b in range(2):
                conv_unit(b, 3, 2)
                tail(b, 3, 5)
            rwTv1 = stt_tile(1)

            cast_tile(3, fmfs3)

            passB_tile(1, rwTv1)

            pool_tile(3)
            # conv rows 5-7 (needs pooled rows 4-7 + zero halo), W-up 0-6/0-7
            for b in range(2):
                conv_unit(b, 5, 2)
                conv_unit(b, 7, 1)
                tail(b, 5, 8)
            rwTv2 = stt_tile(2)
            rwTv3 = stt_tile(3)

            passB_tile(2, rwTv2)
            passB_tile(3, rwTv3)
    if compile:
        nc.compile()
    return nc


def _get_program():
    if "nc" not in _cache:
        _cache["nc"] = _build_program()
    return _cache["nc"]


def kernel(feature_map, kernel, bias):
    from concourse.bass_utils import run_bass_kernel_spmd

    feature_map = np.ascontiguousarray(feature_map, dtype=np.float32)
    kernel = np.ascontiguousarray(kernel, dtype=np.float32)
    bias = np.ascontiguousarray(bias, dtype=np.float32)
    B = feature_map.shape[0]
    assert B == 8

    poolw, hup, kt, bias2, _, _ = _host_consts(kernel, bias)
    nc = _get_program()
    in_maps = [
        {
            "feature_map": feature_map[b],
            "poolw": poolw,
            "hup": hup,
            "ktaps": kt,
            "bias2": bias2,
        }
        for b in range(B)
    ]
    res = run_bass_kernel_spmd(nc, in_maps, list(range(B)))
    out = np.stack([res.results[b]["out"] for b in range(B)])
    return out


# revision 14
# speedup vs baseline: 1.1150x; 1.1150x over previous
"""Trainium2 Bass kernel for nn_InpaintContextAttentionUnit.

Per-sample computation (B=8 samples -> 1 per NeuronCore):
  fm [512,512,16] -> avgpool(64x2) -> pooled [8,256,16]
  -> two masked 3x3 convs (middle row / middle col of kernel zeroed) + bias + relu
  -> bilinear upsample back to [512,512,16] (separable; half-pixel centers, edge clamp)
  -> out [512,512,48] = concat(fm, fm - row_up, fm - col_up)

Design v6 — fully pipelined, upsample entirely on PE, no bf16 staging:
  - H-up AND W-up fused in one matmul: partitions hold 3 j-shifted
    copies of each pooled conv row (filled by DMA from the conv bounce,
    so the shifts are free), lhsT = w_par[j] * hup[n,y] host consts.
    Same PE column count as H-up alone (taps live in the contraction),
    and the DVE W-up STTs (measured 4.7-6.7us each) disappear.
  - j-stack is a PER-TILE rotating tile (bufs=2): no cross-tile
    write-after-read hazards (a shared stack would serialize fills
    behind DMA-paced matmuls of the previous output tile)
  - fm input quarters stay f32 in SBUF until consumed (pass-through
    copy and subtracts read them directly): no bf16 cast pass at all;
    GpSimd only memsets (it measured ~4x slower than DVE on bulk ops)
  - per-engine steady state per 27.7us output tile: DVE ~22us (16 subs
    + 4 pair-adds), ACT ~16us (copies/psum-casts/relus), PE ~16us,
    so the kernel is DMA-bound end to end; out tile t streams from
    ~26us (it needs only pooled conv rows < 2t+3)
  - pooling: W-pair-add + (x,c)->(c,xp) transpose per quarter on DVE,
    one [128,2]x[128,1024] PE matmul per quarter
All constant matrices are precomputed on host and passed as extra inputs.
"""

import numpy as np
import ml_dtypes

H, W, C, F = 512, 512, 16, 16
NPOOL = 8
WP = W // 2  # 256
CH_OUT = 3 * C  # 48

_cache = {}


def _host_consts(kernel, bias):
    """Build host-side constant matrices (bf16 for the PE-side constants)."""
    bf = ml_dtypes.bfloat16
    # pooling weights: [128, 2], 1/128 (exact in bf16) where row block matches
    poolw = np.zeros((128, 2), np.float32)
    poolw[:64, 0] = 1.0 / 128.0
    poolw[64:, 1] = 1.0 / 128.0
    # raw H-upsample matrix: hup[n, y] = weight of pooled row n for output
    # row y (half-pixel centers, edge clamp); all values dyadic -> exact bf16
    hup = np.zeros((NPOOL, H), np.float32)
    scale = H // NPOOL
    for y in range(H):
        yf = (y + 0.5) / scale - 0.5
        i0 = int(np.floor(yf))
        w = yf - i0
        hup[min(max(i0, 0), NPOOL - 1), y] += 1.0 - w
        hup[min(max(i0 + 1, 0), NPOOL - 1), y] += w
    # fused H+W upsample weights: out[y, 2k+par] =
    #   sum_{n,j} wpar[par][j] * hup[n,y] * padc[n, k+j]
    # padc[n, k] = conv[n, k-1] edge-clamped (the j-shifted stack rows)
    # layout: partition 8j + n (branch 0), 32 + 8j + n (branch 1);
    # cols (par, y); rows 24-31 stay zero (also used as the DMA zero src)
    wpar = [[0.25, 0.75, 0.0], [0.0, 0.75, 0.25]]
    hupj = np.zeros((56, 2 * H), np.float32)
    for par in range(2):
        for j in range(3):
            for n in range(NPOOL):
                hupj[8 * j + n, H * par:H * (par + 1)] = wpar[par][j] * hup[n]
    hupj[32:56] = hupj[0:24]
    # conv taps, stacked over dwp on 48 partitions (partition 16g+c holds the
    # dwp=g-1 shifted pooled copy). One matmul slot per (branch, dn):
    #   branch 0 (row conv): kernel[dn+1, dwp+1]; slots 0,1 for dn=-1,+1
    #   branch 1 (col conv): kernel[dwp+1, dn+1], dwp=0 block zero; slots 2-4
    kt = np.zeros((48, 5 * 16), np.float32)  # [(g,c), slot*16+f]
    for g in range(3):
        for s, dn in enumerate((-1, 1)):
            kt[16 * g:16 * (g + 1), s * 16:(s + 1) * 16] = kernel[dn + 1, g]
        for s, dn in enumerate((-1, 0, 1)):
            if g != 1:
                kt[16 * g:16 * (g + 1), (2 + s) * 16:(3 + s) * 16] = \
                    kernel[g, dn + 1]
    bias2 = np.ascontiguousarray(bias.reshape(16, 1)).astype(np.float32)
    return (poolw.astype(bf), hupj.astype(bf), kt.astype(bf), bias2, None, None)


def _build_program(compile=True):
    import concourse.bass as bass
    import concourse.bacc as bacc
    import concourse.mybir as mybir
    import concourse.tile as tile

    dt = mybir.dt.float32
    db = mybir.dt.bfloat16
    nc = bacc.Bacc()

    fm_d = nc.declare_dram_parameter("feature_map", [H, W, C], dt, isOutput=False)
    poolw_d = nc.declare_dram_parameter("poolw", [128, 2], db, isOutput=False)
    hup_d = nc.declare_dram_parameter("hup", [56, 2 * H], db, isOutput=False)
    ktaps_d = nc.declare_dram_parameter("ktaps", [48, 80], db, isOutput=False)
    bias_d = nc.declare_dram_parameter("bias2", [16, 1], dt, isOutput=False)
    out_d = nc.declare_dram_parameter("out", [H, W, CH_OUT], dt, isOutput=True)

    # matmul slots per branch: (slot, dn)
    slots_by_branch = [[(0, -1), (1, 1)], [(2, -1), (3, 0), (4, 1)]]

    with tile.TileContext(nc) as tc:
        with (
            tc.tile_pool(name="consts", bufs=1) as cpool,
            tc.tile_pool(name="persist", bufs=1) as ppool,
            tc.tile_pool(name="work", bufs=1) as wpool,
            tc.tile_pool(name="dram", bufs=1, space="DRAM") as dpool,
            tc.tile_pool(name="psall", bufs=1, space="PSUM") as psall,
        ):
            poolw_t = cpool.tile([128, 2], db)
            hupj_t = cpool.tile([56, 2 * H], db)
            ktaps_t = cpool.tile([48, 80], db)
            bias_t = cpool.tile([16, 1], dt)

            tpad_t = ppool.tile([48, 10 * 258], db)
            t48 = tpad_t[:].rearrange("p (n w) -> p n w", w=258)
            conv_t = ppool.tile([16, 2 * NPOOL * WP], db)

            ncw_dram = dpool.tile([NPOOL, 16 * 258], db)
            nd3 = ncw_dram[:].rearrange("n (c w) -> n c w", w=258)
            ncwd3 = ncw_dram[:].rearrange("n (c w) -> c n w", w=258)
            conv_dram = dpool.tile([16, 2 * NPOOL * WP], db)
            cd4 = conv_dram[:].rearrange("f (b n w) -> b n f w", b=2, n=NPOOL)
            zsrc = hup_d[24:32, 0:16]  # [8, 16] zeros

            # zero-fill the t48 halo rows (0 and 9)
            nc.gpsimd.memset(tpad_t[:], 0.0)

            def load_tile(t):
                fmfs = []
                for qu in range(4):
                    fmf = wpool.tile([128, W * C // 4], dt, tag="fmf", bufs=8,
                                     name=f"fmf{t}{qu}")
                    fmf3 = fmf[:].rearrange("p (x c) -> p x c", c=C)
                    nc.sync.dma_start(
                        out=fmf3,
                        in_=fm_d[128 * t:128 * (t + 1),
                                 128 * qu:128 * (qu + 1)])
                    fmfs.append(fmf)
                return fmfs

            def pool_tile(t, fmfs):
                # per quarter: W-pair add fused with the (x,c)->(c,xp)
                # transpose on DVE, one [128p,2]x[128p,1024] PE matmul,
                # f32->bf16 psum cast on ACT
                fmpT = wpool.tile([128, 16 * WP], db, tag="fmpT", bufs=1,
                                  name=f"fmpT{t}")
                fmpT3 = fmpT[:].rearrange("p (c xp) -> p c xp", xp=WP)
                stage = wpool.tile([2, WP * 16], db, tag="stage", bufs=1,
                                   name=f"stage{t}")
                stage3 = stage[:].rearrange("p (c xp) -> p c xp", xp=WP)
                for qu in range(4):
                    fqp = fmfs[qu][:].rearrange(
                        "p (xp par c) -> p par c xp", par=2, c=16)
                    nc.vector.tensor_add(
                        fmpT3[:, :, 64 * qu:64 * (qu + 1)],
                        fqp[:, 0, :, :], fqp[:, 1, :, :])
                    for hh in range(2):
                        ps = psall.tile([2, 512], dt, tag="pool", bufs=2,
                                        name=f"psp{t}{qu}{hh}")
                        nc.tensor.matmul(
                            ps[:], poolw_t[:],
                            fmpT3[:, 8 * hh:8 * (hh + 1),
                                  64 * qu:64 * (qu + 1)],
                            start=True, stop=True,
                        )
                        nc.scalar.activation(
                            out=stage3[:, 8 * hh:8 * (hh + 1),
                                       64 * qu:64 * (qu + 1)], in_=ps[:],
                            func=mybir.ActivationFunctionType.Copy)
                # bounce to DRAM ((c, xp)-major); read back 3 dwp-shifted
                # copies with c on partitions (n rows shifted +1: zero halo)
                nc.sync.dma_start(
                    out=nd3[2 * t:2 * t + 2, :, 1:257], in_=stage3)
                for g in range(3):
                    nc.sync.dma_start(
                        out=t48[16 * g:16 * (g + 1),
                                2 * t + 1:2 * t + 3, 1:257],
                        in_=ncwd3[:, 2 * t:2 * t + 2, g:g + 256])

            def conv_unit(b, n0, nn):
                # conv rows n0..n0+nn; dwp taps contracted via the
                # 48-partition stack, one accumulating matmul per dn
                ps = psall.tile([16, 2 * WP], dt, tag="conv", bufs=2,
                                name=f"psc{b}{n0}")
                slots = slots_by_branch[b]
                for k, (sl, dn) in enumerate(slots):
                    nc.tensor.matmul(
                        ps[:, 0:nn * WP],
                        ktaps_t[:, sl * 16:(sl + 1) * 16],
                        t48[:, n0 + dn + 1:n0 + dn + 1 + nn, 1:257],
                        start=(k == 0), stop=(k == len(slots) - 1),
                    )
                nc.scalar.activation(
                    out=conv_t[:, (b * NPOOL + n0) * WP:
                               (b * NPOOL + n0 + nn) * WP],
                    in_=ps[:, 0:nn * WP],
                    func=mybir.ActivationFunctionType.Relu,
                    bias=bias_t[:, 0:1],
                )

            def tail(b, nlo, nhi):
                # bounce conv rows [nlo,nhi) to DRAM (read back j-shifted
                # by the per-tile stack fills)
                nc.sync.dma_start(
                    out=conv_dram[:, (b * NPOOL + nlo) * WP:
                                  (b * NPOOL + nhi) * WP],
                    in_=conv_t[:, (b * NPOOL + nlo) * WP:
                               (b * NPOOL + nhi) * WP])

            def rjs_tile(t):
                # per-tile j-shifted conv stack: partition 32b + 8j + n
                # holds padc[b][n, k+j] = conv[b][n, k+j-1] (edge-clamped),
                # k in [0,256). Rows n >= nhi are zero-weighted in hupj;
                # memset the first two allocations so they hold valid
                # (non-NaN) bf16 -- later rotations inherit old valid data.
                nhi = min(8, 2 * t + 3)
                rjs = wpool.tile([56, 16 * 256], db, tag="rjs", bufs=2,
                                 name=f"rjs{t}")
                rjs3 = rjs[:].rearrange("p (f k) -> p f k", k=256)
                if t < 2:
                    nc.gpsimd.memset(rjs[:], 0.0)
                for b in range(2):
                    pg = 32 * b
                    src = cd4[b][0:nhi]
                    for j in range(3):
                        rows = slice(pg + 8 * j, pg + 8 * j + nhi)
                        if j == 0:
                            nc.sync.dma_start(
                                out=rjs3[rows, :, 1:256], in_=src[:, :, 0:255])
                            nc.sync.dma_start(
                                out=rjs3[rows, :, 0:1], in_=src[:, :, 0:1])
                        elif j == 1:
                            nc.sync.dma_start(
                                out=rjs3[rows, :, 0:256], in_=src)
                        else:
                            nc.sync.dma_start(
                                out=rjs3[rows, :, 0:255], in_=src[:, :, 1:256])
                            nc.sync.dma_start(
                                out=rjs3[rows, :, 255:256],
                                in_=src[:, :, 255:256])
                return rjs3

            def passB_tile(t, fmfs, rjs3):
                # output rows 128t..128t+127; contraction span [0, 16+nhi)
                # covers j-blocks 0,1 fully and j=2 rows < nhi (hupj rows
                # n >= nhi are zero); psum holds (f16, k64) per (q,b,par)
                nhi = min(8, 2 * t + 3)
                span = 16 + nhi
                for q in range(4):
                    outq = wpool.tile([128, 128 * CH_OUT], dt,
                                      tag="outq", bufs=3, name=f"oq{t}{q}")
                    outq3 = outq[:].rearrange("p (x ch) -> p x ch",
                                              ch=CH_OUT)
                    outq5 = outq[:].rearrange("p (xp par ch) -> p xp par ch",
                                              par=2, ch=CH_OUT)
                    nc.scalar.activation(
                        out=outq3[:, :, 0:16],
                        in_=fmfs[q][:].rearrange("p (x c) -> p x c", c=16),
                        func=mybir.ActivationFunctionType.Copy,
                    )
                    fq5 = fmfs[q][:].rearrange(
                        "p (xp par c) -> p xp par c", par=2, c=16)
                    for b in range(2):
                        pg = 32 * b
                        for par in range(2):
                            lhsT = hupj_t[pg:pg + span,
                                          H * par + 128 * t:
                                          H * par + 128 * (t + 1)]
                            ps = psall.tile([128, 1024], dt, tag="up",
                                            bufs=2, name=f"psu{t}{q}{b}{par}")
                            for fh in range(2):
                                nc.tensor.matmul(
                                    ps[:, 512 * fh:512 * (fh + 1)],
                                    lhsT,
                                    rjs3[pg:pg + span, 8 * fh:8 * (fh + 1),
                                         64 * q:64 * (q + 1)],
                                    start=True, stop=True,
                                )
                            psx = ps[:].rearrange("p (f k) -> p k f", f=16)
                            nc.vector.tensor_sub(
                                outq5[:, :, par, 16 * (b + 1):16 * (b + 2)],
                                fq5[:, :, par, :], psx)
                    nc.sync.dma_start(
                        out=out_d[128 * t:128 * (t + 1),
                                  128 * q:128 * (q + 1), :],
                        in_=outq3,
                    )

            # ---------------- pipelined schedule ----------------
            fmfs0 = load_tile(0)
            nc.sync.dma_start(out=poolw_t[:], in_=poolw_d[:])
            nc.sync.dma_start(out=hupj_t[:], in_=hup_d[:])
            nc.sync.dma_start(out=ktaps_t[:], in_=ktaps_d[:])
            nc.sync.dma_start(out=bias_t[:], in_=bias_d[:])
            nc.sync.dma_start(out=nd3[:, :, 0:1], in_=zsrc)
            nc.sync.dma_start(out=nd3[:, :, 257:258], in_=zsrc)
            fmfs1 = load_tile(1)

            pool_tile(0, fmfs0)
            # conv row 0 only needs pooled rows 0-1 (tile 0)
            for b in range(2):
                conv_unit(b, 0, 1)
            pool_tile(1, fmfs1)

            # conv rows 1-2 (needs pooled rows 0-3)
            for b in range(2):
                conv_unit(b, 1, 2)
                tail(b, 0, 3)
            rjs0 = rjs_tile(0)

            passB_tile(0, fmfs0, rjs0)

            fmfs2 = load_tile(2)
            pool_tile(2, fmfs2)
            # conv rows 3-4 (needs pooled rows 2-5)
            for b in range(2):
                conv_unit(b, 3, 2)
                tail(b, 3, 5)
            rjs1 = rjs_tile(1)

            passB_tile(1, fmfs1, rjs1)

            fmfs3 = load_tile(3)
            pool_tile(3, fmfs3)
            # conv rows 5-7 (needs pooled rows 4-7 + zero halo)
            for b in range(2):
                conv_unit(b, 5, 2)
                conv_unit(b, 7, 1)
                tail(b, 5, 8)
            rjs2 = rjs_tile(2)
            rjs3v = rjs_tile(3)

            passB_tile(2, fmfs2, rjs2)
            passB_tile(3, fmfs3, rjs3v)
    if compile:
        nc.compile()
    return nc


def _get_program():
    if "nc" not in _cache:
        _cache["nc"] = _build_program()
    return _cache["nc"]


def kernel(feature_map, kernel, bias):
    from concourse.bass_utils import run_bass_kernel_spmd

    feature_map = np.ascontiguousarray(feature_map, dtype=np.float32)
    kernel = np.ascontiguousarray(kernel, dtype=np.float32)
    bias = np.ascontiguousarray(bias, dtype=np.float32)
    B = feature_map.shape[0]
    assert B == 8

    poolw, hup, kt, bias2, _, _ = _host_consts(kernel, bias)
    nc = _get_program()
    in_maps = [
        {
            "feature_map": feature_map[b],
            "poolw": poolw,
            "hup": hup,
            "ktaps": kt,
            "bias2": bias2,
        }
        for b in range(B)
    ]
    res = run_bass_kernel_spmd(nc, in_maps, list(range(B)))
    out = np.stack([res.results[b]["out"] for b in range(B)])
    return out


# revision 15
# speedup vs baseline: 1.3232x; 1.1867x over previous
"""Trainium2 Bass kernel for nn_InpaintContextAttentionUnit.

Per-sample computation (B=8 samples -> 1 per NeuronCore):
  fm [512,512,16] -> avgpool(64x2) -> pooled [8,256,16]
  -> two masked 3x3 convs (middle row / middle col of kernel zeroed) + bias + relu
  -> bilinear upsample back to [512,512,16] (separable; half-pixel centers, edge clamp)
  -> out [512,512,48] = concat(fm, fm - row_up, fm - col_up)

Design v6 — fully pipelined, upsample entirely on PE, no bf16 staging:
  - H-up AND W-up fused in one matmul: partitions hold 3 j-shifted
    copies of each pooled conv row (filled by DMA from the conv bounce,
    so the shifts are free), lhsT = w_par[j] * hup[n,y] host consts.
    Same PE column count as H-up alone (taps live in the contraction),
    and the DVE W-up STTs (measured 4.7-6.7us each) disappear.
  - j-stack is a PER-TILE rotating tile (bufs=2): no cross-tile
    write-after-read hazards (a shared stack would serialize fills
    behind DMA-paced matmuls of the previous output tile)
  - fm input quarters stay f32 in SBUF until consumed (pass-through
    copy and subtracts read them directly): no bf16 cast pass at all;
    GpSimd only memsets (it measured ~4x slower than DVE on bulk ops)
  - per-engine steady state per 27.7us output tile: DVE ~22us (16 subs
    + 4 pair-adds), ACT ~16us (copies/psum-casts/relus), PE ~16us,
    so the kernel is DMA-bound end to end; out tile t streams from
    ~26us (it needs only pooled conv rows < 2t+3)
  - pooling: W-pair-add + (x,c)->(c,xp) transpose per quarter on DVE,
    one [128,2]x[128,1024] PE matmul per quarter
All constant matrices are precomputed on host and passed as extra inputs.
"""

import numpy as np
import ml_dtypes

H, W, C, F = 512, 512, 16, 16
NPOOL = 8
WP = W // 2  # 256
CH_OUT = 3 * C  # 48

_cache = {}


def _host_consts(kernel, bias):
    """Build host-side constant matrices (bf16 for the PE-side constants)."""
    bf = ml_dtypes.bfloat16
    # pooling weights: [128, 2], 1/128 (exact in bf16) where row block matches
    poolw = np.zeros((128, 2), np.float32)
    poolw[:64, 0] = 1.0 / 128.0
    poolw[64:, 1] = 1.0 / 128.0
    # raw H-upsample matrix: hup[n, y] = weight of pooled row n for output
    # row y (half-pixel centers, edge clamp); all values dyadic -> exact bf16
    hup = np.zeros((NPOOL, H), np.float32)
    scale = H // NPOOL
    for y in range(H):
        yf = (y + 0.5) / scale - 0.5
        i0 = int(np.floor(yf))
        w = yf - i0
        hup[min(max(i0, 0), NPOOL - 1), y] += 1.0 - w
        hup[min(max(i0 + 1, 0), NPOOL - 1), y] += w
    # fused H+W upsample weights: out[y, 2k+par] =
    #   sum_{n,j} wpar[par][j] * hup[n,y] * padc[n, k+j]
    # padc[n, k] = conv[n, k-1] edge-clamped (the j-shifted stack rows)
    # layout: partition 8j + n (branch 0), 32 + 8j + n (branch 1);
    # cols (par, y); rows 24-31 stay zero (also used as the DMA zero src)
    wpar = [[0.25, 0.75, 0.0], [0.0, 0.75, 0.25]]
    hupj = np.zeros((56, 2 * H), np.float32)
    for par in range(2):
        for j in range(3):
            for n in range(NPOOL):
                hupj[8 * j + n, H * par:H * (par + 1)] = wpar[par][j] * hup[n]
    hupj[32:56] = hupj[0:24]
    # conv taps, stacked over dwp on 48 partitions (partition 16g+c holds the
    # dwp=g-1 shifted pooled copy). One matmul slot per (branch, dn):
    #   branch 0 (row conv): kernel[dn+1, dwp+1]; slots 0,1 for dn=-1,+1
    #   branch 1 (col conv): kernel[dwp+1, dn+1], dwp=0 block zero; slots 2-4
    kt = np.zeros((48, 5 * 16), np.float32)  # [(g,c), slot*16+f]
    for g in range(3):
        for s, dn in enumerate((-1, 1)):
            kt[16 * g:16 * (g + 1), s * 16:(s + 1) * 16] = kernel[dn + 1, g]
        for s, dn in enumerate((-1, 0, 1)):
            if g != 1:
                kt[16 * g:16 * (g + 1), (2 + s) * 16:(3 + s) * 16] = \
                    kernel[g, dn + 1]
    bias2 = np.ascontiguousarray(bias.reshape(16, 1)).astype(np.float32)
    return (poolw.astype(bf), hupj.astype(bf), kt.astype(bf), bias2, None, None)


def _build_program(compile=True):
    import concourse.bass as bass
    import concourse.bacc as bacc
    import concourse.mybir as mybir
    import concourse.tile as tile

    dt = mybir.dt.float32
    db = mybir.dt.bfloat16
    nc = bacc.Bacc()

    fm_d = nc.declare_dram_parameter("feature_map", [H, W, C], dt, isOutput=False)
    poolw_d = nc.declare_dram_parameter("poolw", [128, 2], db, isOutput=False)
    hup_d = nc.declare_dram_parameter("hup", [56, 2 * H], db, isOutput=False)
    ktaps_d = nc.declare_dram_parameter("ktaps", [48, 80], db, isOutput=False)
    bias_d = nc.declare_dram_parameter("bias2", [16, 1], dt, isOutput=False)
    out_d = nc.declare_dram_parameter("out", [H, W, CH_OUT], dt, isOutput=True)

    # matmul slots per branch: (slot, dn)
    slots_by_branch = [[(0, -1), (1, 1)], [(2, -1), (3, 0), (4, 1)]]

    with tile.TileContext(nc) as tc:
        with (
            tc.tile_pool(name="consts", bufs=1) as cpool,
            tc.tile_pool(name="persist", bufs=1) as ppool,
            tc.tile_pool(name="work", bufs=1) as wpool,
            tc.tile_pool(name="dram", bufs=1, space="DRAM") as dpool,
            tc.tile_pool(name="psall", bufs=1, space="PSUM") as psall,
        ):
            poolw_t = cpool.tile([128, 2], db)
            hupj_t = cpool.tile([56, 2 * H], db)
            ktaps_t = cpool.tile([48, 80], db)
            bias_t = cpool.tile([16, 1], dt)

            tpad_t = ppool.tile([48, 10 * 258], db)
            t48 = tpad_t[:].rearrange("p (n w) -> p n w", w=258)
            conv_t = ppool.tile([16, 2 * NPOOL * WP], db)

            ncw_dram = dpool.tile([NPOOL, 16 * 258], db)
            nd3 = ncw_dram[:].rearrange("n (c w) -> n c w", w=258)
            ncwd3 = ncw_dram[:].rearrange("n (c w) -> c n w", w=258)
            conv_dram = dpool.tile([16, 2 * NPOOL * WP], db)
            cd4 = conv_dram[:].rearrange("f (b n w) -> b n f w", b=2, n=NPOOL)
            zsrc = hup_d[24:32, 0:16]  # [8, 16] zeros

            # zero-fill the t48 halo rows (0 and 9)
            nc.gpsimd.memset(tpad_t[:], 0.0)

            def load_tile(t):
                fmfs = []
                for qu in range(4):
                    fmf = wpool.tile([128, W * C // 4], dt, tag="fmf", bufs=12,
                                     name=f"fmf{t}{qu}")
                    fmf3 = fmf[:].rearrange("p (x c) -> p x c", c=C)
                    nc.gpsimd.dma_start(
                        out=fmf3,
                        in_=fm_d[128 * t:128 * (t + 1),
                                 128 * qu:128 * (qu + 1)])
                    fmfs.append(fmf)
                return fmfs

            def pool_tile(t, fmfs):
                # per quarter: W-pair add fused with the (x,c)->(c,xp)
                # transpose on DVE, one [128p,2]x[128p,1024] PE matmul,
                # f32->bf16 psum cast on ACT
                fmpT = wpool.tile([128, 16 * WP], db, tag="fmpT", bufs=1,
                                  name=f"fmpT{t}")
                fmpT3 = fmpT[:].rearrange("p (c xp) -> p c xp", xp=WP)
                stage = wpool.tile([2, WP * 16], db, tag="stage", bufs=1,
                                   name=f"stage{t}")
                stage3 = stage[:].rearrange("p (c xp) -> p c xp", xp=WP)
                for qu in range(4):
                    fqp = fmfs[qu][:].rearrange(
                        "p (xp par c) -> p par c xp", par=2, c=16)
                    peng = nc.vector if qu < 2 else nc.gpsimd
                    peng.tensor_add(
                        fmpT3[:, :, 64 * qu:64 * (qu + 1)],
                        fqp[:, 0, :, :], fqp[:, 1, :, :])
                    for hh in range(2):
                        ps = psall.tile([2, 512], dt, tag="pool", bufs=2,
                                        name=f"psp{t}{qu}{hh}")
                        nc.tensor.matmul(
                            ps[:], poolw_t[:],
                            fmpT3[:, 8 * hh:8 * (hh + 1),
                                  64 * qu:64 * (qu + 1)],
                            start=True, stop=True,
                        )
                        nc.scalar.activation(
                            out=stage3[:, 8 * hh:8 * (hh + 1),
                                       64 * qu:64 * (qu + 1)], in_=ps[:],
                            func=mybir.ActivationFunctionType.Copy)
                # bounce to DRAM ((c, xp)-major); read back 3 dwp-shifted
                # copies with c on partitions (n rows shifted +1: zero halo)
                nc.scalar.dma_start(
                    out=nd3[2 * t:2 * t + 2, :, 1:257], in_=stage3)
                for g in range(3):
                    nc.scalar.dma_start(
                        out=t48[16 * g:16 * (g + 1),
                                2 * t + 1:2 * t + 3, 1:257],
                        in_=ncwd3[:, 2 * t:2 * t + 2, g:g + 256])

            def conv_unit(b, n0, nn):
                # conv rows n0..n0+nn; dwp taps contracted via the
                # 48-partition stack, one accumulating matmul per dn
                ps = psall.tile([16, 2 * WP], dt, tag="conv", bufs=2,
                                name=f"psc{b}{n0}")
                slots = slots_by_branch[b]
                for k, (sl, dn) in enumerate(slots):
                    nc.tensor.matmul(
                        ps[:, 0:nn * WP],
                        ktaps_t[:, sl * 16:(sl + 1) * 16],
                        t48[:, n0 + dn + 1:n0 + dn + 1 + nn, 1:257],
                        start=(k == 0), stop=(k == len(slots) - 1),
                    )
                nc.scalar.activation(
                    out=conv_t[:, (b * NPOOL + n0) * WP:
                               (b * NPOOL + n0 + nn) * WP],
                    in_=ps[:, 0:nn * WP],
                    func=mybir.ActivationFunctionType.Relu,
                    bias=bias_t[:, 0:1],
                )

            def tail(b, nlo, nhi):
                # bounce conv rows [nlo,nhi) to DRAM (read back j-shifted
                # by the per-tile stack fills)
                nc.gpsimd.dma_start(
                    out=conv_dram[:, (b * NPOOL + nlo) * WP:
                                  (b * NPOOL + nhi) * WP],
                    in_=conv_t[:, (b * NPOOL + nlo) * WP:
                               (b * NPOOL + nhi) * WP])

            def rjs_tile(t):
                # per-tile j-shifted conv stack: partition 32b + 8j + n
                # holds padc[b][n, k+j] = conv[b][n, k+j-1] (edge-clamped),
                # k in [0,256). Rows n >= nhi are zero-weighted in hupj;
                # memset the first two allocations so they hold valid
                # (non-NaN) bf16 -- later rotations inherit old valid data.
                nhi = min(8, 2 * t + 3)
                rjs = wpool.tile([56, 16 * 256], db, tag="rjs", bufs=3,
                                 name=f"rjs{t}")
                rjs3 = rjs[:].rearrange("p (f k) -> p f k", k=256)
                if t < 3:
                    nc.gpsimd.memset(rjs[:], 0.0)
                for b in range(2):
                    pg = 32 * b
                    src = cd4[b][0:nhi]
                    for j in range(3):
                        rows = slice(pg + 8 * j, pg + 8 * j + nhi)
                        if j == 0:
                            nc.gpsimd.dma_start(
                                out=rjs3[rows, :, 1:256], in_=src[:, :, 0:255])
                            nc.gpsimd.dma_start(
                                out=rjs3[rows, :, 0:1], in_=src[:, :, 0:1])
                        elif j == 1:
                            nc.gpsimd.dma_start(
                                out=rjs3[rows, :, 0:256], in_=src)
                        else:
                            nc.gpsimd.dma_start(
                                out=rjs3[rows, :, 0:255], in_=src[:, :, 1:256])
                            nc.gpsimd.dma_start(
                                out=rjs3[rows, :, 255:256],
                                in_=src[:, :, 255:256])
                return rjs3

            def passB_tile(t, fmfs, rjs3):
                # output rows 128t..128t+127; contraction span [0, 16+nhi)
                # covers j-blocks 0,1 fully and j=2 rows < nhi (hupj rows
                # n >= nhi are zero); psum holds (f16, k64) per (q,b,par)
                nhi = min(8, 2 * t + 3)
                span = 16 + nhi
                for q in range(4):
                    outq = wpool.tile([128, 128 * CH_OUT], dt,
                                      tag="outq", bufs=2, name=f"oq{t}{q}")
                    outq3 = outq[:].rearrange("p (x ch) -> p x ch",
                                              ch=CH_OUT)
                    outq5 = outq[:].rearrange("p (xp par ch) -> p xp par ch",
                                              par=2, ch=CH_OUT)
                    nc.scalar.activation(
                        out=outq3[:, :, 0:16],
                        in_=fmfs[q][:].rearrange("p (x c) -> p x c", c=16),
                        func=mybir.ActivationFunctionType.Copy,
                    )
                    fq5 = fmfs[q][:].rearrange(
                        "p (xp par c) -> p xp par c", par=2, c=16)
                    for b in range(2):
                        pg = 32 * b
                        for par in range(2):
                            lhsT = hupj_t[pg:pg + span,
                                          H * par + 128 * t:
                                          H * par + 128 * (t + 1)]
                            ps = psall.tile([128, 1024], dt, tag="up",
                                            bufs=2, name=f"psu{t}{q}{b}{par}")
                            for fh in range(2):
                                nc.tensor.matmul(
                                    ps[:, 512 * fh:512 * (fh + 1)],
                                    lhsT,
                                    rjs3[pg:pg + span, 8 * fh:8 * (fh + 1),
                                         64 * q:64 * (q + 1)],
                                    start=True, stop=True,
                                )
                            psx = ps[:].rearrange("p (f k) -> p k f", f=16)
                            nc.vector.tensor_sub(
                                outq5[:, :, par, 16 * (b + 1):16 * (b + 2)],
                                fq5[:, :, par, :], psx)
                    nc.sync.dma_start(
                        out=out_d[128 * t:128 * (t + 1),
                                  128 * q:128 * (q + 1), :],
                        in_=outq3,
                    )

            # ---------------- pipelined schedule ----------------
            fmfs0 = load_tile(0)
            nc.sync.dma_start(out=poolw_t[:], in_=poolw_d[:])
            nc.sync.dma_start(out=hupj_t[:], in_=hup_d[:])
            nc.sync.dma_start(out=ktaps_t[:], in_=ktaps_d[:])
            nc.sync.dma_start(out=bias_t[:], in_=bias_d[:])
            nc.sync.dma_start(out=nd3[:, :, 0:1], in_=zsrc)
            nc.sync.dma_start(out=nd3[:, :, 257:258], in_=zsrc)
            fmfs1 = load_tile(1)

            fmfs2 = load_tile(2)

            pool_tile(0, fmfs0)
            # conv row 0 only needs pooled rows 0-1 (tile 0)
            for b in range(2):
                conv_unit(b, 0, 1)
            pool_tile(1, fmfs1)

            # conv rows 1-2 (needs pooled rows 0-3)
            for b in range(2):
                conv_unit(b, 1, 2)
                tail(b, 0, 3)
            rjs0 = rjs_tile(0)

            # tile-1 prep floats ahead of the tile-0 output stream
            pool_tile(2, fmfs2)
            # conv rows 3-4 (needs pooled rows 2-5)
            for b in range(2):
                conv_unit(b, 3, 2)
                tail(b, 3, 5)
            rjs1 = rjs_tile(1)

            passB_tile(0, fmfs0, rjs0)

            fmfs3 = load_tile(3)
            pool_tile(3, fmfs3)
            # conv rows 5-7 (needs pooled rows 4-7 + zero halo)
            for b in range(2):
                conv_unit(b, 5, 2)
                conv_unit(b, 7, 1)
                tail(b, 5, 8)
            rjs2 = rjs_tile(2)
            rjs3v = rjs_tile(3)

            passB_tile(1, fmfs1, rjs1)
            passB_tile(2, fmfs2, rjs2)
            passB_tile(3, fmfs3, rjs3v)
    if compile:
        nc.compile()
    return nc


def _get_program():
    if "nc" not in _cache:
        _cache["nc"] = _build_program()
    return _cache["nc"]


def kernel(feature_map, kernel, bias):
    from concourse.bass_utils import run_bass_kernel_spmd

    feature_map = np.ascontiguousarray(feature_map, dtype=np.float32)
    kernel = np.ascontiguousarray(kernel, dtype=np.float32)
    bias = np.ascontiguousarray(bias, dtype=np.float32)
    B = feature_map.shape[0]
    assert B == 8

    poolw, hup, kt, bias2, _, _ = _host_consts(kernel, bias)
    nc = _get_program()
    in_maps = [
        {
            "feature_map": feature_map[b],
            "poolw": poolw,
            "hup": hup,
            "ktaps": kt,
            "bias2": bias2,
        }
        for b in range(B)
    ]
    res = run_bass_kernel_spmd(nc, in_maps, list(range(B)))
    out = np.stack([res.results[b]["out"] for b in range(B)])
    return out
